# revision 25
# baseline (speedup 1.0000x reference)
"""Trainium2 Bass kernel for nn_DenseConv2d_full (dense_cnn).

Computation per sample b (8 samples, data-parallel over 8 NeuronCores):
  step 1: x[(ci,cr), y, w] = sum_{dy<16, dx<8} resp[cr,dy,dx] * imp[ci, y-dy, w-dx]
          (causal / top-left-cropped full conv)
  step 2: y[oc, y, w] = bias[oc] + sum_{(ci,cr), my, mx} conv_w[oc,(ci,cr),my,mx]
          * x[(ci,cr), y+my-1, w+mx-1]   (3x3 SAME conv)

Device mapping (matmul operands bf16, accumulation f32 in PSUM). The PE
cost on this platform is ~N_out_cols * 0.42ns per matmul regardless of K/M,
with a p-state ramp that only reaches max speed after ~3us of CONTINUOUS
execution — so the whole kernel is organized as one software-pipelined
stream that never lets the PE idle:

  Per 16-row block period p:
    - slab prefetch for block p+2: one contiguous 532KB HBM load of 32
      padded impulse rows into partitions 0-31 + three column-shifted
      SBUF->SBUF copies building the dx-shift partition groups (the second
      dx group of 4 is a free rhs column-window offset).
    - phase A of block p: per ci 2 accumulating matmuls
      [K=128=(4 dx-shifts x 32 rows), M=128=(16 y)x(8 cr), N=512] with
      Toeplitz-banded response weights; PSUM evacuated bf16 by DVE/ACT
      column halves; one 2MB x store to a DRAM scratch.
    - row-tile prefetch for block p-1: gather x rows into (ci*8+cr)
      partition order, 2 rows per tile.
    - phase B rows of block p-2 via input-row scatter packing: psum bank =
      output row pair (Y at partitions 0-63, Y+1 at 64-127); input row r's
      two taps that target both rows of a bank merge into ONE full-width
      matmul [K=128 ch, M=128=(2 rows x 64 oc), N=512] sharing one rhs
      stream; the two leftover taps per pair are M=64 column-tiled singles.
      12 matmuls per row pair instead of 18. Packs and singles are grouped
      across quads to minimize PE tiling-mode switches. ACT bias-add
      evacuates, y stored bf16 (tolerance absorbs it).
"""

import os
import sys
from contextlib import ExitStack

import numpy as np

for _p in (
    "/root/.axon_site",
    "/root/.axon_site/_ro/trn_rl_repo",
    "/root/.axon_site/_ro/pypackages",
    "/opt/trn_rl_repo",
):
    if os.path.isdir(_p) and _p not in sys.path:
        sys.path.append(_p)

import concourse.bass as bass  # noqa: E402
import concourse.tile as tile  # noqa: E402
from concourse import bacc, mybir  # noqa: E402
from concourse.bass_utils import run_bass_kernel_spmd  # noqa: E402

F32 = mybir.dt.float32
BF16 = mybir.dt.bfloat16

B, CR, KH, KW = 8, 8, 16, 8
CI, H, W = 16, 256, 512
OC = 64
NBLK = H // 16  # 16 blocks of 16 output rows
WPAD = W + 8  # padded impulse row width (8 left-pad cols)
RPAD = H + 16  # padded impulse rows (16 top-pad rows)

_BUILT = {}


def _row_src(x_scr, y):
    """DRAM AP [ci:16, cr:8, x:W]: element = x_scr[blk, 8*yl+cr, ci, x].

    x_scr layout is [blk][(yl,cr) partition][ci][x]; this gathers one x row
    into the (ci*8+cr)-partition order phase B contracts over.
    """
    blk, yl = divmod(y, 16)
    base = x_scr[:]
    off = blk * 128 * CI * W + yl * 8 * CI * W
    return bass.AP(
        tensor=base.tensor,
        offset=base.offset + off,
        ap=[[W, CI], [CI * W, 8], [1, W]],
    )


def _build_nc(epochs=1, phases="AB", bench_mode=False):
    nc = bacc.Bacc(
        "TRN2",
        target_bir_lowering=False,
        debug=False,
        enable_asserts=False,
        num_devices=8,
    )
    ikind = "Internal" if bench_mode else "ExternalInput"
    # imp_pad[row, ci, col]: zero-padded bf16 impulse (16 top rows, 8 left cols)
    imp_pad = nc.dram_tensor("imp_pad", [RPAD, CI, WPAD], BF16, kind=ikind).ap()
    w_toe = nc.dram_tensor("w_toe", [128, 2, 128], BF16, kind=ikind).ap()
    w9 = nc.dram_tensor("w9", [128, 9, OC], BF16, kind=ikind).ap()
    w9p = nc.dram_tensor("w9p", [128, 6, 2 * OC], BF16, kind=ikind).ap()
    bias2 = nc.dram_tensor("bias2", [128, 1], F32, kind=ikind).ap()
    if bench_mode:
        nc.dram_tensor("dummy_in", [1, 1], F32, kind="ExternalInput")
    y_out = nc.dram_tensor(
        "y_out", [OC, H, W], BF16, kind="Internal" if bench_mode else "ExternalOutput"
    ).ap()
    done = (
        nc.dram_tensor("done", [128, 1], F32, kind="ExternalOutput").ap()
        if bench_mode
        else None
    )
    x_scr = nc.dram_tensor("x_scr", [NBLK, 128, CI, W], BF16).ap()

    with tile.TileContext(nc) as tc, ExitStack() as ctx:
        consts = ctx.enter_context(tc.tile_pool(name="consts", bufs=1))
        imp_pool = ctx.enter_context(tc.tile_pool(name="imp", bufs=4))
        xev_pool = ctx.enter_context(tc.tile_pool(name="xev", bufs=3))
        rows_pool = ctx.enter_context(tc.tile_pool(name="rows", bufs=26))
        yout_pool = ctx.enter_context(tc.tile_pool(name="yt", bufs=8))
        psum_pool = ctx.enter_context(tc.tile_pool(name="psum", bufs=8, space="PSUM"))

        wt = consts.tile([128, 2, 128], BF16)
        nc.sync.dma_start(wt[:], w_toe[:])
        w9t = consts.tile([128, 9, OC], BF16)
        nc.sync.dma_start(w9t[:], w9[:])
        w9pt = consts.tile([128, 6, 2 * OC], BF16)
        nc.sync.dma_start(w9pt[:], w9p[:])
        bt = consts.tile([128, 1], F32)
        nc.sync.dma_start(bt[:], bias2[:])
        zrow = consts.tile([128, W + 2], BF16)
        nc.vector.memset(zrow[:], 0.0)

        for _ep in range(epochs):
            _pipeline(nc, tc, locals(), phases)
        if done is not None:
            nc.sync.dma_start(done, bt[:])

    nc.compile()
    return nc


def _pipeline(nc, tc, env, phases="AB"):
    imp_pad, y_out, x_scr = env["imp_pad"], env["y_out"], env["x_scr"]
    imp_pool, xev_pool, rows_pool, yout_pool = (
        env["imp_pool"],
        env["xev_pool"],
        env["rows_pool"],
        env["yout_pool"],
    )
    psum_pool = env["psum_pool"]
    wt, w9t, bt, zrow, _ep = env["wt"], env["w9t"], env["bt"], env["zrow"], env["_ep"]
    w9pt = env["w9pt"]
    WP = 2 * (W + 2)
    HW2 = W // 2
    slabs = {}
    rowtiles = {}

    def prep_slab(blk):
        # slab[(s,u), ci, c] = imp_pad[16*blk+u, ci, s+c]
        # rhs col window for dx-group g is [1+4g, 1+4g+W) (independent of s).
        if not (0 <= blk < NBLK):
            return
        it = imp_pool.tile([128, CI, WPAD], BF16, tag="imp", name=f"it_{_ep}_{blk}")
        nc.sync.dma_start(
            it[0:32],
            bass.AP(
                tensor=imp_pad.tensor,
                offset=imp_pad.offset + blk * 16 * CI * WPAD,
                ap=[[CI * WPAD, 32], [WPAD, CI], [1, WPAD]],
            ),
        )
        shift_eng = (nc.sync, nc.gpsimd, nc.gpsimd)
        for s in (1, 2, 3):
            shift_eng[s - 1].dma_start(
                it[32 * s : 32 * s + 32, :, 0 : WPAD - 3],
                it[0:32, :, s : s + WPAD - 3],
            )
        slabs[blk] = it

    def a_half(blk, half, state):
        if not (0 <= blk < NBLK):
            return
        if half == 0:
            it = slabs.pop(blk)
            xb = xev_pool.tile([128, CI * W], BF16, tag="xe", name=f"xb_{_ep}_{blk}")
            state[blk] = (it, xb)
        else:
            it, xb = state.pop(blk)
        for ci in range(8 * half, 8 * half + 8):
            ps = psum_pool.tile([128, W], F32, tag="ps", name=f"psA_{_ep}_{blk}_{ci}")
            for g in range(2):
                nc.tensor.matmul(
                    ps[:, :],
                    lhsT=wt[:, 1 - g, :],
                    rhs=it[:, ci, 1 + 4 * g : 1 + 4 * g + W],
                    start=(g == 0),
                    stop=(g == 1),
                )
            # evacuation split DVE/ACT by column half keeps pace with the PE
            nc.vector.tensor_copy(xb[:, ci * W : ci * W + HW2], ps[:, 0:HW2])
            nc.scalar.copy(xb[:, ci * W + HW2 : (ci + 1) * W], ps[:, HW2:W])
        if half == 1:
            nc.scalar.dma_start(
                x_scr[blk], xb.rearrange("p (ci x) -> p ci x", ci=CI)
            )

    def prefetch_rows(m):
        # pair tile i of block m holds x rows (16m+2i, 16m+2i+1) at col
        # offsets 1 and W+3, with zero guard cols {0, W+1, W+2, 2W+3}
        if not (0 <= m < NBLK):
            return
        tiles = []
        for i in range(8):
            q = 8 * m + i
            t = rows_pool.tile([128, WP], BF16, tag="row", name=f"row_{_ep}_{q}")
            nc.vector.memset(
                bass.AP(
                    tensor=t.tensor,
                    offset=t.offset,
                    ap=[[WP, 128], [W + 2, 2], [W + 1, 2]],
                ),
                0.0,
            )
            for j, eng in ((0, nc.sync), (1, nc.gpsimd)):
                eng.dma_start(
                    t[:, 1 + j * (W + 2) : 1 + j * (W + 2) + W],
                    _row_src(x_scr, 2 * q + j),
                )
            tiles.append(t)
        rowtiles[m] = tiles

    def row_ref(y):
        # -> (tile, base col) for x row y; zrow for out-of-range
        if y < 0 or y >= H:
            return zrow, 1
        q, j = divmod(y, 2)
        m, i = divmod(q, 8)
        return rowtiles[m][i], 1 + j * (W + 2)

    def b_quad_open(Y0):
        # 4 psum banks, bank k = output rows (Y, Y+1), Y = Y0+2k: row Y on
        # psum partitions 0-63, row Y+1 on 64-127. Input-row scatter packs
        # two taps per matmul wherever both target rows share the bank:
        #   pack from input row Y   (M=128): my=1 -> Y | my=0 -> Y+1
        #   pack from input row Y+1 (M=128): my=2 -> Y | my=1 -> Y+1
        #   single from row Y+2 (M=64, T1):  my=2 -> Y+1
        #   single from row Y-1 (M=64, T0):  my=0 -> Y
        # 12 matmuls per pair instead of 18, and the full-width packs are
        # FWL-eligible on hardware.
        pss, rtss = [], []
        for k in range(4):
            Y = Y0 + 2 * k
            pss.append(
                psum_pool.tile([128, W], F32, tag="ps", name=f"psB_{_ep}_{Y}")
            )
            rtss.append(
                [row_ref(Y - 1), row_ref(Y), row_ref(Y + 1), row_ref(Y + 2)]
            )
        return pss, rtss

    def b_quad_packs(pss, rtss):
        for pe in range(2):  # pack from input row Y+pe
            for mx in range(3):
                lhsT = w9pt[:, pe * 3 + mx, :]
                for k in range(4):
                    t, base = rtss[k][1 + pe]
                    nc.tensor.matmul(
                        pss[k][:, :],
                        lhsT=lhsT,
                        rhs=t[:, base + mx - 1 : base + mx - 1 + W],
                        start=(pe == 0 and mx == 0),
                        stop=False,
                        skip_group_check=True,
                    )

    def b_quad_singles(pss, rtss):
        for su in range(2):  # su=0: row Y+2 -> Y+1 (T1); su=1: row Y-1 -> Y (T0)
            for mx in range(3):
                t9 = 6 + mx if su == 0 else mx
                for k in range(4):
                    t, base = rtss[k][3 if su == 0 else 0]
                    half = 1 - su
                    nc.tensor.matmul(
                        pss[k][64 * half : 64 * half + 64, :],
                        lhsT=w9t[:, t9, :],
                        rhs=t[:, base + mx - 1 : base + mx - 1 + W],
                        start=False,
                        stop=(su == 1 and mx == 2),
                        tile_position=(0, 64 * half),
                        skip_group_check=True,
                    )

    def b_quad_evac(Y0, pss):
        for k in range(4):
            Y = Y0 + 2 * k
            y2 = yout_pool.tile([128, W], BF16, tag="y2", name=f"y2_{_ep}_{Y}")
            nc.scalar.add(y2[:, :], pss[k][:, :], bt[:, :])
            nc.scalar.dma_start(
                y_out[:, Y : Y + 2, :].rearrange("oc h w -> h oc w"),
                y2[:],
            )

    def b_block(m):
        if not (0 <= m < NBLK):
            return
        g1 = b_quad_open(16 * m)
        g2 = b_quad_open(16 * m + 8)
        b_quad_packs(*g1)
        b_quad_packs(*g2)
        b_quad_singles(*g1)
        b_quad_evac(16 * m, g1[0])
        b_quad_singles(*g2)
        b_quad_evac(16 * m + 8, g2[0])
        rowtiles.pop(m - 1, None)

    def a_block(blk):
        st = {}
        a_half(blk, 0, st)
        a_half(blk, 1, st)

    if phases == "AB":
        prep_slab(0)
        prep_slab(1)
        for p in range(NBLK + 2):
            prep_slab(p + 2)
            a_block(p)
            prefetch_rows(p - 1)
            b_block(p - 2)
    elif phases == "A":
        prep_slab(0)
        prep_slab(1)
        for p in range(NBLK):
            prep_slab(p + 2)
            a_block(p)
    else:  # "B"
        for p in range(NBLK + 1):
            prefetch_rows(p)
            b_block(p - 1)


def _host_prep(response, impulse, conv_w, conv_b):
    """Per-sample input prep (pure layout + bf16 cast, no flops)."""
    import ml_dtypes

    bf16 = ml_dtypes.bfloat16
    in_maps = []
    # w9[(ci*8+cr), my*3+mx, oc] = conv_w[oc, ci*8+cr, my, mx]
    w9f = np.ascontiguousarray(conv_w.transpose(1, 2, 3, 0).reshape(128, 3, 3, OC))
    w9 = w9f.reshape(128, 9, OC).astype(bf16)
    # pack weights: w9p[k, pe*3+mx, e*64+oc] = w9f[k, my_e, mx, oc] with
    # my_e = pe+1 for e=0 (-> out row Y) and pe for e=1 (-> out row Y+1)
    w9p = np.empty((128, 2, 3, 2, OC), np.float32)
    for pe in range(2):
        w9p[:, pe, :, 0] = w9f[:, pe + 1]
        w9p[:, pe, :, 1] = w9f[:, pe]
    w9p = w9p.reshape(128, 6, 2 * OC).astype(bf16)
    bias2 = np.tile(conv_b.astype(np.float32), 2).reshape(128, 1)
    # w_toe[(s,u), dxg, yl*8+cr] = resp[cr, yl+16-u, dxg*4+3-s]
    # vectorized: dy[u, yl] = yl + 16 - u  (valid when 0 <= dy < 16)
    u_idx = np.arange(32)[:, None]
    yl_idx = np.arange(16)[None, :]
    dy = yl_idx + 16 - u_idx  # [32, 16]
    valid = (dy >= 0) & (dy < KH)
    for b in range(B):
        imp_pad = np.zeros((RPAD, CI, WPAD), bf16)
        imp_pad[16:, :, 8:] = impulse[b].transpose(1, 0, 2).astype(bf16)
        wt1 = np.zeros((4, 32, 2, 16, 8), np.float32)  # [s, u, dxg, yl, cr]
        resp = response[b]  # [cr, dy, dx]
        for s in range(4):
            for dxg in range(2):
                dx = dxg * 4 + 3 - s
                # [u, yl, cr] = resp[cr, dy[u,yl], dx] where valid
                r = resp[:, np.clip(dy, 0, KH - 1), dx]  # [cr, 32, 16]
                wt1[s, :, dxg] = np.where(
                    valid[None], r, 0.0
                ).transpose(1, 2, 0)
        in_maps.append(
            {
                "imp_pad": imp_pad,
                "w_toe": wt1.reshape(128, 2, 128).astype(bf16),
                "w9": w9,
                "w9p": w9p,
                "bias2": bias2,
            }
        )
    return in_maps


def kernel(response, impulse, conv_w, conv_b, _trace=False):
    response = np.asarray(response, np.float32)
    impulse = np.asarray(impulse, np.float32)
    conv_w = np.asarray(conv_w, np.float32)
    conv_b = np.asarray(conv_b, np.float32)

    if "nc" not in _BUILT:
        _BUILT["nc"] = _build_nc()
    nc = _BUILT["nc"]

    in_maps = _host_prep(response, impulse, conv_w, conv_b)
    res = run_bass_kernel_spmd(nc, in_maps, list(range(B)), trace=_trace)
    out = np.stack(
        [np.asarray(res.results[b]["y_out"]).astype(np.float32) for b in range(B)],
        axis=0,
    )
    if _trace:
        _BUILT["last_exec_time_ns"] = res.exec_time_ns
        _BUILT["last_results"] = res
    return out


if __name__ == "__main__":
    data = np.load(os.path.join(os.path.dirname(__file__), "ref_cache.npz"))
    out = kernel(data["response"], data["impulse"], data["conv_w"], data["conv_b"])
    ref = data["out"]
    err = np.abs(out - ref).max() / np.abs(ref).max()
    print("Relative error:", err)
